# revision 29
# baseline (speedup 1.0000x reference)
"""Trainium2 Bass kernel for nn_Decoder (NeRF-style 9-layer MLP, Softplus(beta=100)).

Strategy (pure data parallel over 8 cores, feature-major layout):
  - activations live in SBUF as z_l = 100 * y_l (scaled softplus outputs), shape
    [features<=100 partitions, points free-dim]; weights are stationary lhsT.
  - matmuls run in float32r (single-pass PE mode, 4x faster than fp32; ~1e-3
    relative noise, tolerance is 2e-2).
  - softplus(u) = relu(u) + h(t),  t = min(sigma(u), 1-sigma(u)),
    h(t) = -ln(1-t) ~= t*(a + b*t)   (minimax fit on [0,0.5], |err| < 4.1e-3
    in scaled units = 4.1e-5 real units)
    -> per layer-tile: one ACT op (Sigmoid, bias rides free) + one fused
       custom-DVE op (8 ALU stages: computes relu(psum+B) + h(min(s,1-s))).
    This replaces the Exp+Ln+clamp triple (2 ACT + 1 DVE) of the naive
    formulation: ACT work halves.
  - optionally (per-layer config) a DVE-only scheme: softplus ~= relu(u) +
    max(0, bL - aL*|u|)   (1-segment PWL cap, |err| ~ 1.3e-3 real) -- no ACT
    op at all for that layer; used to rebalance ACT vs DVE load.
  - skip connection (layer 4): raw input DMA'd into partitions 98:100 of the
    layer-3 output tile; layer-4 weight columns for those rows scaled by 100.
  - layer 8 (100->1): matmul -> ACT Copy (+b8 bias) -> DMA out.
Supertiles are emitted software-pipelined in groups of GRP (layers interleaved
across the group) so each engine's in-order stream never head-of-line blocks
on the serial per-supertile chain.
"""

import numpy as np

import concourse.bass as bass
import concourse.tile as tile
from concourse import bacc, mybir
from concourse import bass_utils
from concourse.bass_interp import get_hw_module

F32 = mybir.dt.float32
F32R = mybir.dt.float32r
ACTF = mybir.ActivationFunctionType

N_CORES = 8
N_TOTAL = 1048576
P = N_TOTAL // N_CORES          # 131072 points per core
DIMS = [2, 100, 100, 100, 98, 100, 100, 100, 100, 1]

# h(t) = -ln(1-t) ~ t*(SIG_A + SIG_B*t) on [0, 0.5] (minimax, scaled units)
SIG_A = 0.92207691
SIG_B = 0.91213407
# h(u) = ln(1+e^-|u|) ~ max(0, LAM_B - LAM_A*|u|) (minimax, scaled units)
LAM_A = 0.239
LAM_B = 0.6215
# h(u) ~ GAU_A * exp(-GAU_B * u^2) (minimax, scaled units, err 0.069)
GAU_A = 0.624
GAU_B = 0.58

# per-layer activation scheme for layers 0..7:
#  's' = sigmoid-corrected (ACT Sigmoid + DVE fuse, err ~4e-5)
#  'L' = DVE-only PWL      (DVE op only,            err ~7.2e-4)
#  'r' = relu on ACT       (ACT op only,            err ~6.9e-3)
#  'G' = relu + gaussian-h on ACT, summed by next matmul (no DVE, err ~6.9e-4)
#        (not allowed at layer 3: the skip-concat layer needs a single tile)
SCHEMES = "LsLsssss"

_DVE_OPS = {}


def _register_dve_op(name, spec_body, reference):
    """Register (once) a custom DVE op; returns the DveOp."""
    if name in _DVE_OPS:
        return _DVE_OPS[name]
    from concourse import dve_ops
    from concourse.dve_spec import Spec, lower, _has_src1
    from concourse.dve_uop import DveOpSpec
    spec = Spec(body=spec_body, reference=reference)
    op = dve_ops.DveOp(name, spec, subdim=False, uops_sha={})
    dve_ops.OPS.append(op)
    dve_ops.CUSTOM_DVE_SPECS[name] = spec
    dve_ops._SUB_OPCODE_FOR_NAME[name] = (
        dve_ops._CUSTOM_DVE_ROW_BASE + len(dve_ops.OPS) - 1
    )
    assert dve_ops._SUB_OPCODE_FOR_NAME[name] < 0x20
    for ver in ("v3", "v4"):
        uops = lower(spec, ver=ver)
        tmp = DveOpSpec(
            name=name,
            opcode=dve_ops.get_dve_sub_opcode(name),
            uops=uops,
            rd1_en=_has_src1(spec),
        )
        op.uops_sha[ver] = tmp.sha(ver)
    _DVE_OPS[name] = op
    return op


def _get_softplus_sig():
    """out = relu(in1 + s0) + t*(s1 + imm2*t), t = min(in0, 1-in0).
    in0 = sigma (SBUF), in1 = psum, s0 = 100*b [P,1], s1 = SIG_A, imm2 = SIG_B.
    """
    from concourse.dve_spec import Src0, Src1, C0, C1, C2, One, maxx, minn

    # relu(u) + h  ==  max(u + h, h)   (h >= 0) -- serial chain fits 6 lanes
    t = minn(Src0, One - Src0)
    h = (t * C2 + C1) * t
    body = maxx((Src1 + C0) + h, h)

    def ref(in0, in1, s0, s1, imm2):
        in0 = in0.astype(np.float32)
        u = in1.astype(np.float32) + s0
        t = np.minimum(in0, np.float32(1.0) - in0)
        return np.maximum(u, 0) + t * (np.float32(s1) + np.float32(imm2) * t)

    return _register_dve_op("SOFTPLUS_SIG_ANT", body, ref)


def _get_softplus_lam():
    """out = relu(u) + max(0, s1 - imm2*|u|), u = in0 + s0.
    in0 = psum, s0 = 100*b [P,1], s1 = LAM_B, imm2 = LAM_A."""
    from concourse.dve_spec import Src0, C0, C1, C2, Zero, maxx

    u = Src0 + C0
    au = maxx(u, Zero - u)
    body = maxx(u, Zero) + maxx(C1 - C2 * au, Zero)

    def ref(in0, in1, s0, s1, imm2):
        u = in0.astype(np.float32) + s0
        au = np.abs(u)
        return np.maximum(u, 0) + np.maximum(
            np.float32(s1) - np.float32(imm2) * au, np.float32(0.0)
        )

    return _register_dve_op("SOFTPLUS_LAM_ANT", body, ref)


_TABLES_PATCHED = False


def _patch_act_tables():
    """Restrict Sigmoid/Relu/Copy to the sigmoid_and_others table set so the
    table-load placement pass keeps one set loaded for the whole kernel."""
    global _TABLES_PATCHED
    if _TABLES_PATCHED:
        return
    import concourse.hw_specs as hw_specs
    import concourse.bacc as bacc_mod

    orig = hw_specs.get_activation_tables

    def patched(module_arch):
        # Pin Sigmoid and Exp each to a single set so the placement pass
        # never oscillates between equivalent sets. Relu/Square/Identity/Copy
        # exist in every set and are left alone (they never force a switch).
        tables = {k: set(v) for k, v in orig(module_arch).items()}
        for name, funcs in tables.items():
            if name != "sigmoid_and_others":
                funcs.discard(ACTF.Sigmoid)
            if name != "exp_and_others":
                funcs.discard(ACTF.Exp)
        return tables

    hw_specs.get_activation_tables = patched
    bacc_mod.get_activation_tables = patched
    _TABLES_PATCHED = True


def _build_program(T=1024, psum_bufs=4, sbufs=8, mbufs=10, GRP=4, xbufs=6,
                   schemes=SCHEMES, l8_engine="act", fp32_layers=()):
    """fp32_layers: layers whose matmul runs in exact fp32 (4 cycles/row on
    the PE vs 1 for f32r; fine while the PE has slack). Layer-l matmul dtype
    applies to its lhsT and its rhs (= layer l-1's output tile)."""
    NT = P // T
    _patch_act_tables()
    sp_sig = _get_softplus_sig()
    sp_lam = _get_softplus_lam()
    mmdt = [F32 if l in fp32_layers else F32R for l in range(9)]
    nc = bacc.Bacc(
        "TRN2",
        target_bir_lowering=False,
        debug=False,
        enable_asserts=False,
        num_devices=N_CORES,
    )

    # DRAM I/O (per core)
    xt_d = nc.dram_tensor("xt", [2, P], F32, kind="ExternalInput")
    lhsT_d = []
    bias_d = []
    for l in range(9):
        in_dim = 100 if l == 4 else DIMS[l]
        out_dim = DIMS[l + 1]
        lhsT_d.append(
            nc.dram_tensor(f"lhsT{l}", [in_dim, out_dim], F32, kind="ExternalInput")
        )
        if l < 8:
            bias_d.append(
                nc.dram_tensor(f"bias{l}", [out_dim, 1], F32, kind="ExternalInput")
            )
    bsq_d = [
        nc.dram_tensor(f"biassq{l}", [DIMS[l + 1], 1], F32, kind="ExternalInput")
        for l in range(8)
    ]
    # A-scaled weights for the q-part rhs of layers following a 'G' layer
    lhsTq_d = {}
    for l in range(1, 9):
        if schemes[l - 1] == "G":
            in_dim = 100 if l == 4 else DIMS[l]
            lhsTq_d[l] = nc.dram_tensor(
                f"lhsTq{l}", [in_dim, DIMS[l + 1]], F32, kind="ExternalInput"
            )
    b8_d = nc.dram_tensor("b8", [1, 1], F32, kind="ExternalInput")
    y_d = nc.dram_tensor("y", [1, P], F32, kind="ExternalOutput")

    with tile.TileContext(nc) as tc:
        with (
            tc.tile_pool(name="wpool", bufs=1) as wpool,
            tc.tile_pool(name="xpool", bufs=xbufs) as xpool,
            tc.tile_pool(name="psum", bufs=psum_bufs, space="PSUM") as pspool,
            tc.tile_pool(name="spool", bufs=sbufs) as spool,
            tc.tile_pool(name="mpool", bufs=mbufs) as mpool,
            tc.tile_pool(name="qpool", bufs=6) as qpool,
            tc.tile_pool(name="opool", bufs=4) as opool,
        ):
            # --- preload weights/biases ---
            wts = []
            bts = []
            for l in range(9):
                in_dim = 100 if l == 4 else DIMS[l]
                out_dim = DIMS[l + 1]
                wt = wpool.tile([in_dim, out_dim], mmdt[l], tag=f"w{l}")
                nc.sync.dma_start(wt[:], lhsT_d[l].ap().bitcast(mmdt[l]))
                wts.append(wt)
                if l < 8:
                    bt = wpool.tile([out_dim, 1], F32, tag=f"b{l}")
                    nc.sync.dma_start(bt[:], bias_d[l].ap())
                    bts.append(bt)
            bsqts = []
            for l in range(8):
                if schemes[l] == "G":
                    bq = wpool.tile([DIMS[l + 1], 1], F32, tag=f"bq{l}")
                    nc.sync.dma_start(bq[:], bsq_d[l].ap())
                    bsqts.append(bq)
                else:
                    bsqts.append(None)
            wqts = {}
            for l, d in lhsTq_d.items():
                in_dim = 100 if l == 4 else DIMS[l]
                wq = wpool.tile([in_dim, DIMS[l + 1]], mmdt[l], tag=f"wq{l}")
                nc.sync.dma_start(wq[:], d.ap().bitcast(mmdt[l]))
                wqts[l] = wq
            b8t = wpool.tile([1, 1], F32, tag="b8")
            nc.sync.dma_start(b8t[:], b8_d.ap())

            # --- main loop ---
            assert NT % GRP == 0

            def emit_l8(sls, prevs):
                for i in range(GRP):
                    ps = pspool.tile([100, T], F32, tag="ps")
                    parts = prevs[i]
                    ws = [wts[8]] if len(parts) == 1 else [wts[8], wqts[8]]
                    for j in range(T // 512):
                        js = bass.ts(j, 512)
                        for p, part in enumerate(parts):
                            nc.tensor.matmul(
                                ps[0:1, js], ws[p][:], part[0:100, js],
                                start=(p == 0), stop=(p == len(parts) - 1),
                            )
                    out_t = opool.tile([1, T], F32, tag="out")
                    if l8_engine == "act":
                        nc.scalar.activation(
                            out_t[:], ps[0:1, :], ACTF.Identity, bias=b8t[0:1, 0:1],
                        )
                    else:
                        nc.vector.tensor_scalar_add(out_t[:], ps[0:1, :], b8t[0:1, 0:1])
                    nc.sync.dma_start(y_d.ap()[:, sls[i]], out_t[:])

            deferred = None
            for g in range(NT // GRP):
                ts_ids = [g * GRP + i for i in range(GRP)]
                sls = [bass.ts(t, T) for t in ts_ids]
                xts = []
                for sl in sls:
                    xt = xpool.tile([2, T], mmdt[0], tag="xt")
                    nc.sync.dma_start(xt[:], xt_d.ap()[:, sl].bitcast(mmdt[0]))
                    xts.append(xt)
                prevs = [[xt] for xt in xts]
                for l in range(8):
                    if l == 2 and deferred is not None:
                        emit_l8(*deferred)
                        deferred = None
                    in_dim = 100 if l == 4 else DIMS[l]
                    out_dim = DIMS[l + 1]
                    sch = schemes[l]
                    pss = []
                    for i in range(GRP):
                        ps = pspool.tile([100, T], F32, tag="ps")
                        parts = prevs[i]
                        ws = [wts[l]] if len(parts) == 1 else [wts[l], wqts[l]]
                        for j in range(T // 512):
                            js = bass.ts(j, 512)
                            for p, part in enumerate(parts):
                                nc.tensor.matmul(
                                    ps[0:out_dim, js],
                                    ws[p][:],
                                    part[0:in_dim, js],
                                    start=(p == 0),
                                    stop=(p == len(parts) - 1),
                                )
                        pss.append(ps)
                    sigs = []
                    if sch == "s":
                        for i in range(GRP):
                            sg = spool.tile([100, T], F32, tag="sg")
                            nc.scalar.activation(
                                sg[0:out_dim, :], pss[i][0:out_dim, :], ACTF.Sigmoid,
                                bias=bts[l][:, 0:1],
                            )
                            sigs.append(sg)
                    nprevs = []
                    for i in range(GRP):
                        if sch == "G":
                            assert l != 3, "G not allowed at the skip-concat layer"
                            r = mpool.tile([100, T], mmdt[l + 1], tag="m7" if l == 7 else "m")
                            sq = spool.tile([100, T], F32, tag="sg")
                            q = qpool.tile([100, T], mmdt[l + 1], tag="q7" if l == 7 else "q")
                            nc.scalar.activation(
                                r[0:out_dim, :].bitcast(F32), pss[i][0:out_dim, :],
                                ACTF.Relu, bias=bts[l][:, 0:1],
                            )
                            nc.scalar.activation(
                                sq[0:out_dim, :], pss[i][0:out_dim, :],
                                ACTF.Square, bias=bsqts[l][:, 0:1],
                                scale=float(np.sqrt(GAU_B)),
                            )
                            nc.scalar.activation(
                                q[0:out_dim, :].bitcast(F32), sq[0:out_dim, :],
                                ACTF.Exp, scale=-1.0,
                            )
                            nprevs.append([r, q])
                            continue
                        m = mpool.tile([100, T], mmdt[l + 1], tag="m7" if l == 7 else "m")
                        if l == 3:
                            nc.sync.dma_start(
                                m[98:100, :], xt_d.ap()[:, sls[i]].bitcast(mmdt[4])
                            )
                        if sch == "s":
                            nc.vector._custom_dve(
                                sp_sig,
                                out=m[0:out_dim, :],
                                in0=sigs[i][0:out_dim, :],
                                in1=pss[i][0:out_dim, :],
                                s0=bts[l][:, 0:1],
                                s1=SIG_A,
                                imm2=SIG_B,
                            )
                        elif sch == "L":
                            nc.vector._custom_dve(
                                sp_lam,
                                out=m[0:out_dim, :],
                                in0=pss[i][0:out_dim, :],
                                s0=bts[l][:, 0:1],
                                s1=LAM_B,
                                imm2=LAM_A,
                            )
                        else:  # 'r'
                            nc.scalar.activation(
                                m[0:out_dim, :].bitcast(F32),
                                pss[i][0:out_dim, :],
                                ACTF.Relu,
                                bias=bts[l][:, 0:1],
                            )
                        nprevs.append([m])
                    prevs = nprevs
                if deferred is not None:
                    emit_l8(*deferred)
                deferred = (sls, prevs)
            if deferred is not None:
                emit_l8(*deferred)

    nc.compile()
    nc.m = get_hw_module(nc.m)
    return nc


def _transform_weights(inputs):
    """Host-side weight/bias transform -> per-program DRAM tensors (shared
    across cores)."""
    W = [np.asarray(inputs[f"W{l}"], dtype=np.float32) for l in range(9)]
    b = [np.asarray(inputs[f"b{l}"], dtype=np.float32) for l in range(9)]
    t = {}
    t["lhsT0"] = np.ascontiguousarray((100.0 * W[0]).T)
    for l in (1, 2, 3, 5, 6, 7):
        t[f"lhsT{l}"] = np.ascontiguousarray(W[l].T)
    t["lhsT4"] = np.ascontiguousarray(
        np.concatenate([W[4][:, 2:].T, (100.0 * W[4][:, :2]).T], axis=0)
    )
    t["lhsT8"] = np.ascontiguousarray(W[8].T / 100.0)
    for l in range(8):
        t[f"bias{l}"] = np.ascontiguousarray((100.0 * b[l]).reshape(-1, 1))
        t[f"biassq{l}"] = np.ascontiguousarray(
            (np.sqrt(GAU_B) * 100.0 * b[l]).astype(np.float32).reshape(-1, 1)
        )
    for l in range(8):
        if SCHEMES[l] == "G":
            t[f"lhsTq{l + 1}"] = np.ascontiguousarray(
                (GAU_A * t[f"lhsT{l + 1}"]).astype(np.float32)
            )
    t["b8"] = np.ascontiguousarray(b[8].reshape(1, 1))
    return t


_NC_CACHE = None


def kernel(**inputs) -> np.ndarray:
    global _NC_CACHE
    if _NC_CACHE is None:
        _NC_CACHE = _build_program()
    nc = _NC_CACHE

    x = np.asarray(inputs["input"], dtype=np.float32)
    assert x.shape == (N_TOTAL, 2)
    shared = _transform_weights(inputs)

    in_maps = []
    for c in range(N_CORES):
        m = dict(shared)
        m["xt"] = np.ascontiguousarray(x[c * P : (c + 1) * P].T)
        in_maps.append(m)

    res = bass_utils.run_bass_kernel_spmd(nc, in_maps, core_ids=list(range(N_CORES)))
    y = np.concatenate([res.results[c]["y"][0] for c in range(N_CORES)])
    return y.reshape(N_TOTAL, 1).astype(np.float32)
